# revision 17
# baseline (speedup 1.0000x reference)
"""Thole dipole-dipole interaction tensor kernel for Trainium2 (8 NeuronCores).

Strategy
--------
The per-edge math is purely elementwise except for the two polarisability
gathers pol[edge_src], pol[edge_dst].  Per-element gathers have no fast path
on TRN2 (indirect-DMA descriptors and GPSIMD lookups are ~100x off the memory
roofline), so the edge list is sorted by destination atom on the host and
sharded by destination-atom range across the 8 cores.  In that order
pol[edge_dst] is a run-length expansion, which the DVE computes at line rate
with a multiplicative-mask prefix scan (tensor_tensor_scan; exact in fp32:
the state is either carried or reset, never accumulated).  The host places
the ~12.5k per-core run-start values into a dense injection plane (the
on-device indirect-DMA scatter only supports one offset per partition row, so
the placement itself is host-side data movement).  The pol[edge_src] gather
is a host-side fancy index feeding a per-edge f32 plane -- its HBM traffic is
identical to shipping edge_src itself.  The output is computed in sorted
order as the 6 unique planes of the symmetric 3x3 tensor; the host mirrors
and unpermutes.

Per-edge math (folded units; B = BOHR):
    au3 = A * d^3 / sqrt(ps*pd)            (unit conversions cancel)
    e = exp(-au3); lam3 = 1-e; lam5 = 1-(1+au3)e
    t[a,b] = 3*lam5*B^3/d^3 * u_a*u_b - lam3*B^3/d^3 * delta_ab,  u = vec/d
with au3 evaluated as A*exp(3*ln d - 0.5*ln(ps*pd)) on the scalar engine and
1/d via the 2-ULP DVE reciprocal.

Layout: all per-edge inputs are packed as planes of one [128, 6, L] tensor
(d, ps, inj, vx, vy, vz) so each tile needs a single 3D-AP DMA; the output is
[128, 6, L] planes (xx, yy, zz, xy, xz, yz).
"""

import os
from contextlib import ExitStack

import numpy as np

P = 128
N_TILES = 20
N_CORES = 8
BOHR = 0.52917721067
A_MUTUAL = 0.39
B3 = float(BOHR**3)
LN_A = float(np.log(np.float64(A_MUTUAL)))

N_ATOMS = 100000
E_TOTAL = 6400000
T = 316
L = N_TILES * T          # 6320
E_PAD = P * L            # 808960 >= max shard size (~801k)

# input plane indices
PD_, PPS, PINJ, PVX, PVY, PVZ = range(6)


def build_nc(mode="scan", bench_loops=1):
    """Build the Bass program. mode: "scan" (on-device dst gather) or
    "hostpd" (pd supplied per edge by the host, for validation)."""
    import concourse.bass as bass
    import concourse.tile as tile
    from concourse import bacc, mybir

    dt = mybir.dt
    op = mybir.AluOpType
    act = mybir.ActivationFunctionType

    nc = bacc.Bacc("TRN2", target_bir_lowering=False)

    pack = nc.dram_tensor("pack", [P, 6, L], dt.float32, kind="ExternalInput")
    if mode == "hostpd":
        pd_in = nc.dram_tensor("pd", [P, L], dt.float32, kind="ExternalInput")
    out = nc.dram_tensor("t_out", [P, 6, L], dt.float32, kind="ExternalOutput")

    with tile.TileContext(nc) as tc, ExitStack() as ctx:
        io_pool = ctx.enter_context(tc.tile_pool(name="io", bufs=3))
        mid_pool = ctx.enter_context(tc.tile_pool(name="mid", bufs=2))
        out_pool = ctx.enter_context(tc.tile_pool(name="out", bufs=3))
        once_pool = ctx.enter_context(tc.tile_pool(name="once", bufs=1))

        c_ln_a = once_pool.tile([P, 1], dt.float32)
        nc.vector.memset(c_ln_a[:], LN_A)

        for _ in range(bench_loops):
            carry = None
            for t in range(N_TILES):
                ts_ = bass.ts(t, T)
                pk = io_pool.tile([P, 6 * T], dt.float32, tag="pk")
                nc.sync.dma_start(
                    pk[:].rearrange("p (c t) -> p c t", c=6), pack[:, :, ts_]
                )

                def pv(c):
                    return pk[:, c * T:(c + 1) * T]

                d = pv(PD_)
                p_s = pv(PPS)

                if mode == "hostpd":
                    p_d_t = io_pool.tile([P, T], dt.float32, tag="pd")
                    nc.scalar.dma_start(p_d_t[:], pd_in[:, ts_])
                    p_d = p_d_t[:]
                else:
                    s1 = pv(PINJ)
                    # carry mask: 1 where not a run start (injection == 0)
                    d0 = mid_pool.tile([P, T], dt.float32, tag="d0")
                    nc.vector.tensor_scalar(d0[:], s1, 0.0, None, op.is_equal)
                    p_d_t = mid_pool.tile([P, T], dt.float32, tag="pd")
                    nc.vector.tensor_tensor_scan(
                        p_d_t[:], d0[:], s1,
                        0.0 if carry is None else carry,
                        op.mult, op.add,
                    )
                    carry = p_d_t[:, T - 1:T]
                    p_d = p_d_t[:]

                alpha = mid_pool.tile([P, T], dt.float32, tag="alpha")
                nc.vector.tensor_tensor(alpha[:], p_s, p_d, op.mult)

                la = mid_pool.tile([P, T], dt.float32, tag="la")
                nc.scalar.activation(la[:], alpha[:], act.Ln)
                ld = mid_pool.tile([P, T], dt.float32, tag="ld")
                nc.scalar.activation(ld[:], d, act.Ln)
                ld3 = mid_pool.tile([P, T], dt.float32, tag="ld3")
                nc.vector.tensor_scalar(ld3[:], ld[:], 3.0, None, op.mult)
                z = mid_pool.tile([P, T], dt.float32, tag="z")
                nc.vector.scalar_tensor_tensor(z[:], la[:], -0.5, ld3[:],
                                               op.mult, op.add)
                au3 = mid_pool.tile([P, T], dt.float32, tag="au3")
                nc.scalar.activation(au3[:], z[:], act.Exp, bias=c_ln_a[:])
                e = mid_pool.tile([P, T], dt.float32, tag="e")
                nc.scalar.activation(e[:], au3[:], act.Exp, scale=-1.0)
                p5 = mid_pool.tile([P, T], dt.float32, tag="p5")
                nc.vector.scalar_tensor_tensor(p5[:], au3[:], 1.0, e[:],
                                               op.add, op.mult)
                lam5 = mid_pool.tile([P, T], dt.float32, tag="lam5")
                nc.vector.tensor_scalar(lam5[:], p5[:], -1.0, 1.0,
                                        op.mult, op.add)
                lam3 = mid_pool.tile([P, T], dt.float32, tag="lam3")
                nc.scalar.activation(lam3[:], e[:], act.Identity,
                                     bias=1.0, scale=-1.0)

                # ird = 1/d (2-ULP); g = B^3/d^3
                ird = mid_pool.tile([P, T], dt.float32, tag="ird")
                rscr = mid_pool.tile([P, T], dt.float32, tag="rscr")
                nc.vector.reciprocal_approx_accurate(ird[:], d, rscr[:])
                ird2 = mid_pool.tile([P, T], dt.float32, tag="ird2")
                nc.scalar.activation(ird2[:], ird[:], act.Square)
                g = mid_pool.tile([P, T], dt.float32, tag="g")
                nc.vector.scalar_tensor_tensor(g[:], ird2[:], B3, ird[:],
                                               op.mult, op.mult)

                c5e = mid_pool.tile([P, T], dt.float32, tag="c5e")
                nc.vector.scalar_tensor_tensor(c5e[:], lam5[:], 3.0, g[:],
                                               op.mult, op.mult)
                s_ = mid_pool.tile([P, T], dt.float32, tag="s_")
                nc.scalar.activation(s_[:], c5e[:], act.Sqrt)
                c3 = mid_pool.tile([P, T], dt.float32, tag="c3")
                nc.gpsimd.tensor_tensor(c3[:], lam3[:], g[:], op.mult)

                u = []
                for a, eng in ((PVX, "gps"), (PVY, "gps"), (PVZ, "dve")):
                    ua = mid_pool.tile([P, T], dt.float32, tag=f"u{a}")
                    if eng == "gps":
                        nc.gpsimd.tensor_tensor(ua[:], pv(a), ird[:], op.mult)
                    else:
                        nc.vector.tensor_tensor(ua[:], pv(a), ird[:], op.mult)
                    u.append(ua)
                y = []
                for a in range(3):
                    ya = mid_pool.tile([P, T], dt.float32, tag=f"y{a}")
                    nc.vector.tensor_tensor(ya[:], u[a][:], s_[:], op.mult)
                    y.append(ya)

                o = out_pool.tile([P, 6 * T], dt.float32, tag="o")

                def ov(c):
                    return o[:, c * T:(c + 1) * T]

                for a in range(3):
                    nc.scalar.activation(ov(a), y[a][:], act.Square)
                    nc.gpsimd.tensor_tensor(ov(a), ov(a), c3[:], op.subtract)
                nc.vector.tensor_tensor(ov(3), y[0][:], y[1][:], op.mult)
                nc.vector.tensor_tensor(ov(4), y[0][:], y[2][:], op.mult)
                nc.vector.tensor_tensor(ov(5), y[1][:], y[2][:], op.mult)

                nc.sync.dma_start(
                    out[:, :, ts_], o[:].rearrange("p (c t) -> p c t", c=6)
                )

    nc.compile()
    return nc


def _host_prep(edge_src, edge_dst, distances, vec, polarisability):
    """Sort edges by dst, shard by dst-atom range, build per-core packed
    input planes."""
    f32 = np.float32
    E = edge_src.shape[0]
    atoms_per_core = N_ATOMS // N_CORES

    order = np.argsort(edge_dst, kind="stable")
    dst_sorted = edge_dst[order]
    bounds = np.searchsorted(
        dst_sorted, np.arange(1, N_CORES) * atoms_per_core
    )
    bounds = np.concatenate([[0], bounds, [E]])

    pol_src_all = polarisability[edge_src]

    in_maps = []
    counts = []
    for c in range(N_CORES):
        lo, hi = bounds[c], bounds[c + 1]
        Ec = int(hi - lo)
        assert Ec <= E_PAD - 1, f"shard {c} too large: {Ec}"
        counts.append(Ec)
        idx = order[lo:hi]
        dst_c = dst_sorted[lo:hi]

        pk = np.empty((6, E_PAD), f32)
        pk[PD_] = 1.0
        pk[PD_, :Ec] = distances[idx]
        pk[PPS] = 1.0
        pk[PPS, :Ec] = pol_src_all[idx]
        pk[PVX:PVZ + 1] = 0.0
        pk[PVX:PVZ + 1, :Ec] = vec[idx].T

        start = np.zeros(E_PAD, bool)
        start[1:Ec] = dst_c[1:] != dst_c[:-1]
        start[0] = True
        start[np.arange(P) * L] = True
        start_pos = np.flatnonzero(start)
        # dense injection plane: pol[dst] at run starts (1.0 in the pad
        # region so pad lanes stay finite), 0 elsewhere
        inj = np.zeros(E_PAD, f32)
        real = start_pos < Ec
        inj[start_pos[real]] = polarisability[dst_c[start_pos[real]]]
        inj[start_pos[~real]] = 1.0
        pk[PINJ] = inj

        # [6, E_PAD] -> [P, 6, L]
        in_maps.append(
            {"pack": np.ascontiguousarray(
                pk.reshape(6, P, L).transpose(1, 0, 2))}
        )
    return in_maps, order, bounds, counts


def kernel(species, edge_src, edge_dst, distances, vec, polarisability):
    from concourse.bass_utils import run_bass_kernel_spmd

    edge_src = np.asarray(edge_src)
    edge_dst = np.asarray(edge_dst)
    distances = np.asarray(distances, dtype=np.float32)
    vec = np.asarray(vec, dtype=np.float32)
    polarisability = np.asarray(polarisability, dtype=np.float32)
    E = edge_src.shape[0]
    assert E == E_TOTAL and polarisability.shape[0] == N_ATOMS

    in_maps, order, bounds, counts = _host_prep(
        edge_src, edge_dst, distances, vec, polarisability
    )

    mode = os.environ.get("KERNEL_MODE", "scan")
    if mode == "hostpd":
        pol_dst_all = polarisability[edge_dst]
        for c in range(N_CORES):
            lo, hi = bounds[c], bounds[c + 1]
            pd_c = np.ones(E_PAD, np.float32)
            pd_c[: counts[c]] = pol_dst_all[order[lo:hi]]
            in_maps[c]["pd"] = pd_c.reshape(P, L)

    nc = build_nc(mode)
    kernel.last_nc = nc
    kernel.last_in_maps = in_maps
    res = run_bass_kernel_spmd(
        nc, in_maps, core_ids=list(range(N_CORES)),
        trace=bool(int(os.environ.get("KERNEL_TRACE", "0"))),
    )

    # planes: xx yy zz xy xz yz  ->  [E, 9]
    out = np.empty((E, 9), np.float32)
    col_of_plane = ((0,), (4,), (8,), (1, 3), (2, 6), (5, 7))
    for c in range(N_CORES):
        lo, hi = bounds[c], bounds[c + 1]
        Ec = counts[c]
        planes = (
            res.results[c]["t_out"].reshape(P, 6, L)
            .transpose(1, 0, 2).reshape(6, E_PAD)[:, :Ec]
        )
        t9 = np.empty((Ec, 9), np.float32)
        for pl, cols in enumerate(col_of_plane):
            for col in cols:
                t9[:, col] = planes[pl]
        out[order[lo:hi]] = t9
    kernel.last_results = res
    return out.reshape(E, 3, 3)
